# revision 1
# baseline (speedup 1.0000x reference)
"""Trainium2 Bass kernel for nn_CompletenessLoss (OHEM hinge loss with top-k).

Self-contained: accepts FULL inputs, shards over 8 NeuronCores internally
(data-parallel over the group dimension), returns the full scalar output.

Math (reference):
  scores[i]  = pred[i, labels[i]-1]
  groups of 64 rows: first 8 are "positive", last 56 are "negative"
  pos_ls = sum over all positive rows of relu(1 - s)
  neg_ls = sum over groups of (sum of top-9 of relu(1 + s) over 56 negatives)
  out    = (pos_ls + neg_ls) / (num_pos + int(num_neg * 0.17))

Implementation per core (32768 rows):
  - partition p owns rows [p*256, (p+1)*256) -> every DMA reads a contiguous
    span per partition (max HBM efficiency) and each partition's score row
    contains 4 whole groups along the free axis (no transpose needed).
  - gather: one fused DVE scalar_tensor_tensor per score column block:
      (iota == label[p]) * pred[p, :] summed along free dim -> score
  - hinge on ScalarE; top-9 of negatives via DVE max8 + match_replace + max
"""

import numpy as np

# Problem geometry (hardcoded per the harness contract).
N_FULL = 262144
D = 200                      # pred_dim
GS = 64                      # sample_group_size
SS = 8                       # sample_split (positives per group)
OHEM_RATIO = 0.17
KEEP = int((GS - SS) * OHEM_RATIO)   # 9 hardest negatives kept per group

N_CORES = 8
ROWS = N_FULL // N_CORES     # 32768 rows per core
P = 128                      # SBUF partitions
NTILES = ROWS // P           # 256 rows per partition = 4 groups
CHUNK = 32                   # rows-per-partition per DMA (1.6 MB bf16 per dma_start)
USE_BF16 = True              # cast pred to bf16 on host (halves DMA traffic)

_compiled = None             # cached program so repeat calls skip rebuild


def build_nc(rows=ROWS, chunk=CHUNK, use_bf16=USE_BF16):
    """Build the per-core Bass program. All 8 cores run this same program
    on their own shard (SPMD)."""
    import concourse.bacc as bacc
    import concourse.tile as tile
    from concourse import mybir

    f32 = mybir.dt.float32
    pdt = mybir.dt.bfloat16 if use_bf16 else f32
    ntiles = rows // P
    assert rows % (P * chunk) == 0 and ntiles % GS == 0

    # Bacc (not plain Bass): its compile() runs generate_event_semaphores,
    # which splits multi-sem waits — TRN2 allows 1 wait per instruction.
    nc = bacc.Bacc("TRN2", target_bir_lowering=False, debug=False,
                   num_devices=N_CORES)
    pred_t = nc.dram_tensor("pred", [rows, D], pdt, kind="ExternalInput")
    # labt[p, t] = labels[p*ntiles + t] - 1, as f32
    lab_t = nc.dram_tensor("labt", [P, ntiles], f32, kind="ExternalInput")
    # cio: the 0..D-1 class ramp, host-provided in pred's dtype
    cio_t = nc.dram_tensor("cio", [1, D], pdt, kind="ExternalInput")
    out_t = nc.dram_tensor("partial", [P, 2], f32, kind="ExternalOutput")

    with tile.TileContext(nc) as tc:
        _body(tc, pred_t.ap(), lab_t.ap(), cio_t.ap(), out_t.ap(), ntiles,
              chunk, pdt)
    nc.compile()
    return nc


def _body(tc, pred, labt, cio, out, ntiles, chunk, pdt):
    from concourse import mybir

    nc = tc.nc
    f32 = mybir.dt.float32
    AX = mybir.AxisListType
    OP = mybir.AluOpType
    AF = mybir.ActivationFunctionType

    nchunks = ntiles // chunk
    gpp = ntiles // GS              # groups per partition

    from contextlib import ExitStack
    with ExitStack() as ctx:
        singles = ctx.enter_context(tc.tile_pool(name="singles", bufs=1))
        chunks = ctx.enter_context(tc.tile_pool(name="chunks", bufs=3))
        ph2 = ctx.enter_context(tc.tile_pool(name="ph2", bufs=2))

        # --- one-time constants ---
        # iota ramp comes from the host (broadcast-DMA'd to all partitions)
        # so no gpsimd instruction is needed: gpsimd's first op would pay a
        # ~6us library IRAM load.
        import concourse.bass as bass
        iota = singles.tile([P, D], pdt)
        nc.sync.dma_start(
            out=iota,
            in_=bass.AP(tensor=cio.tensor, offset=cio.offset,
                        ap=[[0, P]] + list(cio.ap)))
        labs = singles.tile([P, ntiles], f32)
        nc.sync.dma_start(out=labs, in_=labt)

        # per-group-slot score staging: stages[g][p, u] = score of row
        # p*ntiles + g*GS + u  — separate tiles so phase 2 for group g can
        # start as soon as its 64 columns are gathered.
        stages = [singles.tile([P, GS], f32, name=f"stage{g}", tag=f"stage{g}")
                  for g in range(gpp)]

        # Prologue: touch iota/labs on DVE so the hot-loop ops only ever
        # wait on their pred-chunk DMA (TRN2: 1 sync wait per instruction).
        warm = singles.tile([P, 1], f32)
        nc.vector.tensor_scalar(out=warm, in0=labs[:, 0:1], scalar1=0.0,
                                scalar2=1.0, op0=OP.mult, op1=OP.mult)
        warm2 = singles.tile([P, 1], pdt)
        nc.vector.tensor_copy(warm2, iota[:, 0:1])

        # --- phase 1: stream pred, gather label-indexed scores ---
        # row index = p*ntiles + t  (contiguous per partition). Small first
        # chunks shorten the DVE ramp-up before the first big DMA lands.
        plan = []
        t0 = 0
        for sz in [4, 4, 8, 16]:
            plan.append((t0, sz))
            t0 += sz
        while t0 < ntiles:
            sz = min(chunk, ntiles - t0)
            plan.append((t0, sz))
            t0 += sz
        pred_v = pred.rearrange("(p t) j -> p t j", p=P)
        for (tbase, sz) in plan:
            ch = chunks.tile([P, chunk, D], pdt, tag="ch")
            nc.sync.dma_start(out=ch[:, 0:sz, :],
                              in_=pred_v[:, tbase:tbase + sz, :])
            for b in range(sz):
                t = tbase + b
                # stage[g][p, u] = sum_j (j==lab[p,t]) * pred_row[j].
                # The product is written in place over the consumed chunk
                # slice — no scratch tile, so no slot-recycle semaphores.
                nc.vector.scalar_tensor_tensor(
                    out=ch[:, b, :], in0=iota, scalar=labs[:, t:t + 1],
                    in1=ch[:, b, :], op0=OP.is_equal, op1=OP.mult,
                    accum_out=stages[t // GS][:, t % GS:t % GS + 1])

        # --- phase 2: per partition, gpp whole groups along the free axis ---
        pp = singles.tile([P, gpp], f32)             # pos sums per group slot
        negacc = singles.tile([P, 2 * gpp], f32)     # top8-sum & 9th cols
        for g in range(gpp):
            stg = stages[g]
            # positives: relu(1 - s), fused sum along free dim
            ptmp = ph2.tile([P, SS], f32, tag="ptmp")
            nc.scalar.activation(
                out=ptmp, in_=stg[:, 0:SS], func=AF.Relu,
                bias=1.0, scale=-1.0, accum_out=pp[:, g:g + 1])
            # negatives: relu(1 + s), then sum of top-9 of 56
            nl = ph2.tile([P, GS - SS], f32, tag="nl")
            nc.scalar.activation(
                out=nl, in_=stg[:, SS:GS],
                func=AF.Relu, bias=1.0, scale=1.0)
            m8 = ph2.tile([P, 8], f32, tag="m8")
            nc.vector.max(out=m8, in_=nl)
            nc.vector.match_replace(
                out=nl, in_to_replace=m8, in_values=nl, imm_value=-1.0)
            nc.vector.tensor_reduce(
                out=negacc[:, 2 * g:2 * g + 1], in_=m8, axis=AX.X, op=OP.add)
            nc.vector.tensor_reduce(
                out=negacc[:, 2 * g + 1:2 * g + 2], in_=nl, axis=AX.X,
                op=OP.max)

        # --- final per-partition reduction -> [P, 2] ---
        res = singles.tile([P, 2], f32)
        nc.vector.tensor_reduce(out=res[:, 0:1], in_=pp, axis=AX.X, op=OP.add)
        nc.vector.tensor_reduce(out=res[:, 1:2], in_=negacc, axis=AX.X,
                                op=OP.add)
        nc.sync.dma_start(out=out, in_=res)


def _get_compiled():
    global _compiled
    if _compiled is None:
        _compiled = build_nc()
    return _compiled


def _prep_core_inputs(pred, labels):
    """Split full inputs into per-core input maps."""
    pred = np.asarray(pred)
    if USE_BF16:
        import ml_dtypes
        pred = pred.astype(ml_dtypes.bfloat16)
    else:
        pred = np.ascontiguousarray(pred.astype(np.float32, copy=False))
    lab = np.asarray(labels).astype(np.int64)
    cio = np.arange(D).reshape(1, D).astype(pred.dtype)
    in_maps = []
    for c in range(N_CORES):
        sl = slice(c * ROWS, (c + 1) * ROWS)
        lab_sh = (lab[sl] - 1).astype(np.float32)
        labt = np.ascontiguousarray(lab_sh.reshape(P, NTILES))
        in_maps.append({"pred": np.ascontiguousarray(pred[sl]), "labt": labt,
                        "cio": cio})
    return in_maps


def _finalize(results):
    pos = 0.0
    neg = 0.0
    for r in results:
        part = r["partial"].astype(np.float64)
        pos += part[:, 0].sum()
        neg += part[:, 1].sum()
    num_pos = (N_FULL // GS) * SS
    num_neg = N_FULL - num_pos
    denom = float(num_pos + int(num_neg * OHEM_RATIO))
    return np.float32((pos + neg) / denom)


def kernel(pred, labels, sample_split, sample_group_size):
    assert int(sample_split) == SS and int(sample_group_size) == GS
    from concourse.bass_utils import run_bass_kernel_spmd

    nc = _get_compiled()
    in_maps = _prep_core_inputs(pred, labels)
    res = run_bass_kernel_spmd(nc, in_maps, core_ids=list(range(N_CORES)))
    return _finalize(res.results)

